# Initial kernel scaffold
#
"""Trainium2 Bass kernel for a 2D DWT (depthwise 8x8 conv, stride 2).

Reference computes a depthwise conv of x [16, 64, 256, 256] with 4 subband
filters that are outer products of an 8-tap low/high pair -> separable:
apply the (low|high) banded filter matrix along H via one matmul pass,
then along W via a second pass.  Output [16, 256, 125, 125] with channel
order [ll(64) | lh(64) | hl(64) | hh(64)].

Design notes (from trace iteration):
- fp16 matmul operands (x cast on host): LDWEIGHTS pipelines behind the
  previous matmul; warm back-to-back rate is ~N/2.4GHz.  fp32/f32r weight
  loads do not pipeline (measured 262+374 ns per pair).
- HWDGE stores with small per-partition chunks (2 KB) get placed on only
  5 of 16 SDMA engines (~130 GB/s); with 16 KB contiguous per partition
  they spread across all 16 (~320 GB/s).  So the output DRAM layout is
  [b, hy, c, s, wx]: for a fixed output row hy, 8 consecutive channels
  are contiguous -> one store per 8 images moves 16 KB per partition.
  Host does the final transpose to [b, s*64+c, hy, wx].
- Input is loaded 2 rows per partition (polyphase over even/odd H) so a
  256-row fp16 image is one DMA of 1 KB-contiguous chunks.
- DMA issue and PSUM->SBUF copies are spread over Sync/Scalar/Vector.

Sharding: pure data parallel over batch, 2 images-per-core x 8 cores.
"""

import numpy as np

B, C, H, W = 16, 64, 256, 256
HP = WP = 125
N_CORES = 8
B_SH = B // N_CORES  # 2 batches per core
GRP = 2  # images per output store

_LOW = np.array(
    [0.1629, 0.5055, 0.4464, -0.0198, -0.1323, 0.0218, 0.0233, -0.0075],
    dtype=np.float32,
)
_HIGH = np.array(
    [-0.0075, -0.0233, 0.0218, 0.1323, -0.0198, -0.4464, 0.5055, -0.1629],
    dtype=np.float32,
)


def _band_matrix() -> np.ndarray:
    """BM[h, f*128 + y] = filt_f[h - 2y] for 0 <= h-2y < 8.

    Columns 125:128 and 253:256 are zero padding so each filter block is
    128 wide (full-width stationary operands, moving free dim 256).
    """
    bm = np.zeros((256, 256), dtype=np.float32)
    for f, filt in enumerate((_LOW, _HIGH)):
        for y in range(125):
            bm[2 * y : 2 * y + 8, f * 128 + y] = filt
    return bm


def _band_consts() -> np.ndarray:
    """[4, 128, 256] fp16: BM even rows, BM odd rows, BM[0:128], BM[128:256]."""
    bm = _band_matrix()
    return np.stack([bm[0::2], bm[1::2], bm[0:128], bm[128:256]]).astype(np.float16)


_CACHE = {}


def _build_bass():
    import concourse.bacc as bacc
    import concourse.mybir as mybir
    from concourse.tile import TileContext

    f32 = mybir.dt.float32
    f16 = mybir.dt.float16

    nc = bacc.Bacc("TRN2")
    x_d = nc.dram_tensor("x", [B_SH, C, H, W], f16, kind="ExternalInput")
    bm_d = nc.dram_tensor("bmc", [4, 128, 256], f16, kind="ExternalInput")
    # [b, c//GRP, hy(128), c%GRP, subband, wx]: each (b, c-group) is one
    # contiguous block with hy outermost.  hy runs to 128 (3 pad rows the
    # host strips): stores sourced from 128 SBUF partitions spread across
    # all 16 SDMA engines, while 125-partition stores land on only 5
    # (measured; partition count is what decides the spread).
    out_d = nc.dram_tensor(
        "out", [B_SH, C // GRP, 128, GRP, 4, WP], f32, kind="ExternalOutput"
    )

    with TileContext(nc) as tc:
        with (
            tc.tile_pool(name="const", bufs=1) as cpool,
            tc.tile_pool(name="xin", bufs=16) as xpool,
            tc.tile_pool(name="asb", bufs=8) as apool,
            tc.tile_pool(name="bsb", bufs=8) as bpool,
            tc.tile_pool(name="aps", bufs=4, space="PSUM") as apspool,
            tc.tile_pool(name="bps", bufs=4, space="PSUM") as bpspool,
        ):
            bm_e = cpool.tile([128, 256], f16, tag="bme")
            bm_o = cpool.tile([128, 256], f16, tag="bmo")
            bm0 = cpool.tile([128, 256], f16, tag="bm0")
            bm1 = cpool.tile([128, 256], f16, tag="bm1")
            nc.sync.dma_start(out=bm_e[:], in_=bm_d[0])
            nc.sync.dma_start(out=bm_o[:], in_=bm_d[1])
            nc.sync.dma_start(out=bm0[:], in_=bm_d[2])
            nc.sync.dma_start(out=bm1[:], in_=bm_d[3])

            for b in range(B_SH):
                for c0 in range(0, C, GRP):
                    # bt holds GRP images: image j at cols [j*500, j*500+500),
                    # inner layout (s, wx) -> 16 KB contiguous DRAM for a
                    # fixed output row hy across GRP consecutive channels.
                    bt = bpool.tile([128, GRP * 500], f32, tag="bt")
                    for j in range(GRP):
                        c = c0 + j
                        # x image as [p, (r w)]: partition p = rows 2p, 2p+1
                        xt = xpool.tile([128, 512], f16, tag="xt")
                        nc.gpsimd.dma_start(
                            out=xt[:],
                            in_=x_d[b, c].rearrange("(p r) w -> p (r w)", r=2),
                        )

                        # Pass A: A[w, f*128+hy] = sum_h x[h,w]*BM[h, col],
                        # h = 2p + r accumulated over even/odd row matmuls.
                        # One accumulation group fills a whole PSUM bank:
                        # w-chunk 0 -> cols 0:256, w-chunk 1 -> cols 256:512
                        # (start clears the bank's has_written bits once, so
                        # chunk 1's first matmul overwrites, second accumulates)
                        a_ps = apspool.tile([128, 512], f32, tag="aps")
                        for wc in range(2):
                            for r in range(2):
                                nc.tensor.matmul(
                                    a_ps[:, wc * 256 : wc * 256 + 256],
                                    xt[:, r * 256 + wc * 128 : r * 256 + wc * 128 + 128],
                                    (bm_e if r == 0 else bm_o)[:],
                                    start=(wc == 0 and r == 0),
                                    stop=(wc == 1 and r == 1),
                                    skip_group_check=True,
                                )
                        a_sb = apool.tile([128, 512], f16, tag="asb")
                        nc.vector.tensor_copy(a_sb[:], a_ps[:])

                        # Pass B: B[hy, g*128+wx] =
                        #   sum_w A[w, f*128+hy] * BM[w, g*128+wx]
                        # fv=0 -> cols 0:256, fv=1 -> cols 256:512
                        b_ps = bpspool.tile([128, 512], f32, tag="bps")
                        for fv in range(2):
                            for wc in range(2):
                                nc.tensor.matmul(
                                    b_ps[:, fv * 256 : fv * 256 + 256],
                                    a_sb[:, wc * 256 + fv * 128 : wc * 256 + fv * 128 + 128],
                                    (bm0 if wc == 0 else bm1)[:],
                                    start=(fv == 0 and wc == 0),
                                    stop=(fv == 1 and wc == 1),
                                    skip_group_check=True,
                                )
                        src = b_ps[:].rearrange("p (v g x) -> p v g x", v=2, g=2)
                        dst = bt[:, j * 500 : j * 500 + 500].rearrange(
                            "p (v g x) -> p v g x", v=2, g=2
                        )
                        nc.scalar.copy(dst, src[:, :, :, 0:125])

                    # one store per GRP images: contiguous 2 MB block
                    out_ap = out_d[b, c0 // GRP].rearrange("h c s w -> h (c s w)")
                    nc.sync.dma_start(out=out_ap, in_=bt[:])
    nc.finalize()
    return nc


def kernel(x: np.ndarray, trace: bool = False):
    from concourse.bass_utils import run_bass_kernel_spmd

    x = np.asarray(x)
    assert x.shape == (B, C, H, W), x.shape
    x16 = np.ascontiguousarray(x.astype(np.float16))

    if "nc" not in _CACHE:
        _CACHE["nc"] = _build_bass()
    nc = _CACHE["nc"]

    bmc = _band_consts()
    in_maps = [
        {"x": x16[i * B_SH : (i + 1) * B_SH], "bmc": bmc} for i in range(N_CORES)
    ]
    res = run_bass_kernel_spmd(
        nc, in_maps, core_ids=list(range(N_CORES)), trace=trace
    )
    # [16, 8, 128, 8, 4, 125] (b, cg, hy+pad, cj, s, wx)
    #   -> strip 3 hy pad rows -> (b, s, cg, cj, hy, wx) -> [16, 256, 125, 125]
    raw = np.concatenate([r["out"] for r in res.results], axis=0)[:, :, :HP]
    out = np.ascontiguousarray(raw.transpose(0, 4, 1, 3, 2, 5)).reshape(
        B, 4 * C, HP, WP
    )
    if trace:
        return out, res
    return out



# revision 58
# speedup vs baseline: 1.3526x; 1.3526x over previous
"""Trainium2 Bass kernel for a 2D DWT (depthwise 8x8 conv, stride 2).

Reference computes a depthwise conv of x [16, 64, 256, 256] with 4 subband
filters that are outer products of an 8-tap low/high pair -> separable:
apply the (low|high) banded filter matrix along H via one matmul pass,
then along W via a second pass.  Output [16, 256, 125, 125] with channel
order [ll(64) | lh(64) | hl(64) | hh(64)].

Design notes (from trace iteration; baseline 158.7us -> ~120us):
- fp16 matmul operands (x cast on host): LDWEIGHTS pipelines fully behind
  back-to-back fp16 matmuls even at N=64 (measured 28ns issue spacing).
- Output stored as fp16 (host casts to f32): halves store traffic (fp32
  stores put the per-core DMA floor at ~138us).
- BOTH passes are "banded": with x stored as row-halves (partition p =
  rows p and p+128), BM[0:128] columns are nonzero only for y 0..63 and
  BM[128:256] only for y 61..124, so each pass is 8 matmuls of N=64
  (512 cycles) instead of 4 of N=256 (1024); the y 61..63 overlap
  accumulates at identical PSUM addresses inside one group.  One [128,512]
  fp16 band-matrix constant serves both passes.
- PSUM->SBUF copies are the pacing stage (~1.0ns/col + ~150-270ns/op on
  DVE/Act, and DVE+Act contend for PSUM reads when overlapped): images are
  processed in PAIRS sharing a 2-bank PSUM tile so one CAST (vector) and
  one strided bt-copy (scalar) cover 2 images.  Pass B of pair k is
  emitted after pass A of pair k+1 (LAG=1 software pipeline) so the CAST
  mostly hides under the next pair's matmuls.
- Loads (gpsimd queue) and stores (sync queue) must be on separate queues
  (head-of-line blocking otherwise); mixed-direction DMA sustains only
  ~330 GB/s of the ~420 single-direction peak, making DMA the overall
  wall.  Load prefetch is throttled (xpool bufs) so store bursts are not
  starved; GRP=4 images per store (4KB/partition chunks) beats 8/16.
- Host pre-transposes x to [b, c-group, p, (c2, half, w)] so each load is
  one plain 2D DMA with 4KB contiguous per partition.

Sharding: pure data parallel over batch, 2 images-per-core x 8 cores.
"""

import numpy as np

B, C, H, W = 16, 64, 256, 256
HP = WP = 125
N_CORES = 8
B_SH = B // N_CORES  # 2 batches per core
GRP = 4  # images per output store (4 KB per partition per store)
BANDED_B = True  # banded pass-B moving slices (N=64 x8 vs N=256 x4)
LGRP = 4  # images per input load DMA

_LOW = np.array(
    [0.1629, 0.5055, 0.4464, -0.0198, -0.1323, 0.0218, 0.0233, -0.0075],
    dtype=np.float32,
)
_HIGH = np.array(
    [-0.0075, -0.0233, 0.0218, 0.1323, -0.0198, -0.4464, 0.5055, -0.1629],
    dtype=np.float32,
)


def _band_matrix() -> np.ndarray:
    """BM[h, f*128 + y] = filt_f[h - 2y] for 0 <= h-2y < 8.

    Columns 125:128 and 253:256 are zero padding so each filter block is
    128 wide (full-width stationary operands, moving free dim 256).
    """
    bm = np.zeros((256, 256), dtype=np.float32)
    for f, filt in enumerate((_LOW, _HIGH)):
        for y in range(125):
            bm[2 * y : 2 * y + 8, f * 128 + y] = filt
    return bm


def _band_consts() -> np.ndarray:
    """[128, 512] fp16: BM[0:128] | BM[128:256] (used by both passes)."""
    bm = _band_matrix()
    return np.concatenate([bm[0:128], bm[128:256]], axis=1).astype(np.float16)


_CACHE = {}


def _build_bass():
    import concourse.bacc as bacc
    import concourse.mybir as mybir
    from concourse.tile import TileContext

    f32 = mybir.dt.float32
    f16 = mybir.dt.float16

    nc = bacc.Bacc("TRN2")
    # x pre-transposed on host to [b, c-group, p, (c2 r w)]: partition p
    # holds rows 2p, 2p+1 of LGRP consecutive channel-images -> each load
    # is a plain 2D DMA with LGRP KB contiguous per partition.
    x_d = nc.dram_tensor(
        "x", [B_SH, C // LGRP, 128, LGRP * 512], f16, kind="ExternalInput"
    )
    bm_d = nc.dram_tensor("bmc", [128, 512], f16, kind="ExternalInput")
    # [b, c//GRP, hy(128), c%GRP, subband, wx]: each (b, c-group) is one
    # contiguous block with hy outermost.  hy runs to 128 (3 pad rows the
    # host strips): stores sourced from 128 SBUF partitions spread across
    # all 16 SDMA engines, while 125-partition stores land on only 5
    # (measured; partition count is what decides the spread).
    out_d = nc.dram_tensor(
        "out", [B_SH, C // GRP, 128, GRP, 4, WP], f16, kind="ExternalOutput"
    )

    with TileContext(nc) as tc:
        with (
            tc.tile_pool(name="const", bufs=1) as cpool,
            tc.tile_pool(name="xin", bufs=6) as xpool,
            tc.tile_pool(name="asb", bufs=6) as apool,
            tc.tile_pool(name="bsb", bufs=4) as bpool,
            tc.tile_pool(name="aps", bufs=2, space="PSUM") as apspool,
            tc.tile_pool(name="bps", bufs=2, space="PSUM") as bpspool,
        ):
            bma = cpool.tile([128, 512], f16, tag="bma")
            nc.scalar.dma_start(out=bma[:], in_=bm_d[:])
            bm0 = bma[:, 0:256]
            bm1 = bma[:, 256:512]

            # Flat pair pipeline, software-pipelined by one pair: the PE
            # stream is A(k), B(k-1), A(k+1), B(k), ... so B never waits
            # for the CAST that feeds it (the CAST runs during the next
            # pair's A matmuls).  Two images per PSUM tile (one bank
            # each); one vector CAST and one scalar bt-copy per pair.
            n_pairs = (B_SH * C) // 2
            ppg = GRP // 2  # pairs per store group
            ppl = LGRP // 2  # pairs per load

            def emit_b(p):
                asb2, bt, j0, store = p
                bps2 = bpspool.tile([128, 1024], f32, tag="bps")
                for h in (0, 512):
                    _pass_b(nc, asb2, h, bps2, bm0, bm1)
                src = bps2[:].rearrange(
                    "p (i v g x) -> p i v g x", i=2, v=2, g=2
                )
                dst = bt[:, j0 * 500 : j0 * 500 + 1000].rearrange(
                    "p (i v g x) -> p i v g x", i=2, v=2, g=2
                )
                nc.scalar.copy(dst, src[:, :, :, :, 0:125])
                if store is not None:
                    nc.sync.dma_start(out=store, in_=bt[:])

            from collections import deque

            LAG = 1
            pending = deque()
            bt = xt = None
            for k in range(n_pairs):
                img0 = k * 2
                b, c = img0 // C, img0 % C
                if k % ppl == 0:
                    xt = xpool.tile([128, LGRP * 512], f16, tag="xt")
                    # first two loads go out on sync/scalar as well-inited
                    # early queues to overlap the per-engine init preamble
                    eng = (nc.sync, nc.scalar, nc.gpsimd)[min(k // ppl, 2)]
                    eng.dma_start(out=xt[:], in_=x_d[b, c // LGRP])
                if k % ppg == 0:
                    bt = bpool.tile([128, GRP * 500], f16, tag="bt")
                aps2 = apspool.tile([128, 1024], f32, tag="aps")
                asb2 = apool.tile([128, 1024], f16, tag="asb")
                jp = (k % ppl) * 2
                for jj in (jp, jp + 1):
                    _pass_a(nc, xt, jj, aps2, (jj - jp) * 512, bm0, bm1)
                nc.vector.tensor_copy(asb2[:], aps2[:])
                store = None
                if k % ppg == ppg - 1:
                    store = out_d[b, c // GRP].rearrange("h c s w -> h (c s w)")
                pending.append((asb2, bt, (k % ppg) * 2, store))
                if len(pending) > LAG:
                    emit_b(pending.popleft())
            while pending:
                emit_b(pending.popleft())
    nc.finalize()
    return nc


def _copy(eng, dst, src):
    if hasattr(eng, "tensor_copy"):
        eng.tensor_copy(dst, src)
    else:
        eng.copy(dst, src)


def _pass_a(nc, xt, jj, aps2, h, bm0, bm1):
    """A[w, f*128+hy] = sum_h x[h,w]*BM[h, f*128+hy], banded.

    xt partition p holds rows p (half 0) and p+128 (half 1), so the
    contraction is over raw rows: BM[0:128] columns are nonzero only for
    hy 0..63 and BM[128:256] only for hy 61..124.  The hy 61..63 overlap
    accumulates at identical PSUM addresses within one group (the group
    confined to this image's bank, cols [h, h+512)).
    """
    x0 = jj * 512
    n = 0
    for wc in range(2):
        for half in range(2):
            st = xt[:, x0 + half * 256 + wc * 128 : x0 + half * 256 + wc * 128 + 128]
            bm = bm0 if half == 0 else bm1
            c0 = 0 if half == 0 else 61
            for f in range(2):
                oc = h + wc * 256 + f * 128 + c0
                nc.tensor.matmul(
                    aps2[:, oc : oc + 64],
                    st,
                    bm[:, f * 128 + c0 : f * 128 + c0 + 64],
                    start=(n == 0),
                    stop=(n == 7),
                    skip_group_check=True,
                )
                n += 1


def _pass_b(nc, asb2, h, bps2, bm0, bm1):
    """B[hy, g*128+wx] = sum_w A[w, f*128+hy] * BM[w, g*128+wx], banded.

    BM[w 0..127] cols are nonzero only for wx 0..63; BM[w 128..255] only
    for wx 61..124; the wx 61..63 overlap accumulates in PSUM.
    fv=0 -> cols h..h+256, fv=1 -> cols h+256..h+512.
    """
    n = 0
    for fv in range(2):
        for wc in range(2):
            st = asb2[:, h + wc * 256 + fv * 128 : h + wc * 256 + fv * 128 + 128]
            for g in range(2):
                if wc == 0:
                    mv = bm0[:, g * 128 : g * 128 + 64]
                    oc = h + fv * 256 + g * 128
                else:
                    mv = bm1[:, g * 128 + 61 : g * 128 + 125]
                    oc = h + fv * 256 + g * 128 + 61
                nc.tensor.matmul(
                    bps2[:, oc : oc + 64],
                    st,
                    mv,
                    start=(n == 0),
                    stop=(n == 7),
                    skip_group_check=True,
                )
                n += 1


def kernel(x: np.ndarray, trace: bool = False):
    from concourse.bass_utils import run_bass_kernel_spmd

    x = np.asarray(x)
    assert x.shape == (B, C, H, W), x.shape
    # [b, c-group, p, c2, half, w]: partition p = rows p, p+128 per image
    x16 = np.ascontiguousarray(
        x.astype(np.float16)
        .reshape(B, C // LGRP, LGRP, 2, H // 2, W)
        .transpose(0, 1, 4, 2, 3, 5)
    )

    if "nc" not in _CACHE:
        _CACHE["nc"] = _build_bass()
    nc = _CACHE["nc"]

    bmc = _band_consts()
    in_maps = [
        {"x": x16[i * B_SH : (i + 1) * B_SH], "bmc": bmc} for i in range(N_CORES)
    ]
    res = run_bass_kernel_spmd(
        nc, in_maps, core_ids=list(range(N_CORES)), trace=trace
    )
    # [16, C//GRP, 128, GRP, 4, 125] (b, cg, hy+pad, cj, s, wx)
    #   -> strip 3 hy pad rows -> (b, s, cg, cj, hy, wx) -> [16, 256, 125, 125]
    raw = np.concatenate([r["out"] for r in res.results], axis=0)[:, :, :HP]
    out = (
        np.ascontiguousarray(raw.transpose(0, 4, 1, 3, 2, 5))
        .reshape(B, 4 * C, HP, WP)
        .astype(np.float32)
    )
    if trace:
        return out, res
    return out
